# revision 24
# baseline (speedup 1.0000x reference)
"""vq_codebook kernel for trn2: cosine-sim argmax over K=65536 codes + gather.

Strategy: shard K across 8 cores (slab Kc=8192 per core). Host pre-normalizes
W columns and t rows, scales by 8, casts both to fp8 e4m3. The device runs:

  - fp8 DoubleRow matmul screen (contraction 256 = 2x128 in ONE pass at
    0.5 cycles/out-col -> 4x the fp16 PE rate): sims*64 -> PSUM fp32.
  - per 128-row block (4 PSUM tiles of 2048):
      tiles 0-2: ACT copies to fp16 SBUF (6144 cols); DVE runs a 4-level
          packed fp16 binary max tree (2x mode) -> 384 stride-384 group roots
      tile 3: DVE tensor_reduce(axis=X, max) straight from PSUM
          -> 128 consecutive-16 group roots
    The drain is ACT/DVE balanced (~5850ns/block each); fp8 keeps the PE
    (~3400ns/block) out of the critical path.
  - roots stream to HBM in segments; host does the argmax.

Host: argmax over the 8*512=4096 group roots per row; the winning group plus
every group whose root is within BAND (fp8 screen noise) of the winner is
rescored exactly (float64) and the best code gathered.
"""

import os
import sys

import numpy as np

for _p in ("/opt/trn_rl_repo", "/root/.axon_site/_ro/trn_rl_repo"):
    if os.path.isdir(_p) and _p not in sys.path:
        sys.path.append(_p)

import ml_dtypes

import concourse.bass as bass  # noqa: F401  (AP types via tile)
import concourse.tile as tile
from concourse import bacc, mybir
from concourse.bass_utils import run_bass_kernel_spmd

P = 128
B, D, K, NCORES = 8192, 256, 65536, 8
KC = K // NCORES        # 8192 per-core codebook slab
G = 16                  # codes per root group
CTREE = 6144            # ACT-copied cols per block (tree part)
CDIR = KC - CTREE       # DVE direct-reduced cols per block (2048)
NTR = CTREE // G        # tree roots per block (384)
NDR = CDIR // G         # direct roots per block (128)
RPB = NTR + NDR         # roots per block (512)
MB = B // P             # row blocks (64)
SCALE = 8.0             # fp8 pre-scale per operand; sims carry SCALE^2
EPS = 1e-7

# fp8 e4m3 screen noise band (cosine units), incl. PE accumulation + fp16
# root cast (measured 1.3e-4) and fp8 quantization error (rms ~1.6e-3,
# measured max 1.27e-2 on a 4M-sim sample). 1.8e-2 = ~40% above the sampled
# max; host-only cost (~1.7x candidate groups), zero device-time impact.
BAND = 1.8e-2

F32 = mybir.dt.float32
F16 = mybir.dt.float16
F8 = mybir.dt.float8e4
AF = mybir.ActivationFunctionType
ALU = mybir.AluOpType
AX = mybir.AxisListType


def build_core_kernel(nc, b=B, d=D, kc=KC):
    assert d == 2 * P and kc == KC
    mb = b // P

    tT = nc.dram_tensor("tT", [P, 2 * b], F8, kind="ExternalInput")
    wn = nc.dram_tensor("wn", [P, 2 * kc], F8, kind="ExternalInput")
    roots_d = nc.dram_tensor("roots", [P, mb * RPB], F16, kind="ExternalOutput")

    with tile.TileContext(nc) as tc:
        with (
            tc.tile_pool(name="persist", bufs=1) as persist,
            tc.tile_pool(name="scopy", bufs=1) as scp,
            tc.tile_pool(name="tree", bufs=1) as trp,
            tc.tile_pool(name="psum", bufs=1, space="PSUM") as psump,
        ):
            Tn = persist.tile([P, 2 * b], F8)      # targ fp8, d-half major
            Wn = persist.tile([P, 2 * kc], F8)     # unit-col W fp8, d-half major
            roots = persist.tile([P, mb * RPB], F16)

            # ---- input DMA: first rows of t, then W tile by tile, then rest
            tpre = 6 * P
            nc.sync.dma_start(out=Tn[:, 0:tpre], in_=tT[:, 0:tpre])
            nc.sync.dma_start(out=Tn[:, b : b + tpre], in_=tT[:, b : b + tpre])
            for q in range(4):  # W quarter by quarter (both d-halves)
                for i in range(2):
                    o = i * kc + q * 2048
                    nc.sync.dma_start(out=Wn[:, o : o + 2048], in_=wn[:, o : o + 2048])
            nc.sync.dma_start(out=Tn[:, tpre:b], in_=tT[:, tpre:b])
            nc.sync.dma_start(out=Tn[:, b + tpre :], in_=tT[:, b + tpre :])

            t3 = Tn[:].rearrange("p (j c) -> p j c", j=2)
            w3 = Wn[:].rearrange("p (j c) -> p j c", j=2)

            # roots DMA segment bounds (blocks), tapered at the end
            ends, e = [], 0
            for w in [8] * (mb // 8 - 1) + [4, 2, 2]:
                e += w
                ends.append(e)
            DMA_BOUNDS = {e1: e0 for e0, e1 in zip([0] + ends[:-1], ends)}

            # Per block: 6 PSUM tiles over 3 fixed slots -- A/B (1536 wide,
            # 3 banks each) for ACT copies, C (1024, 2 banks) for DVE direct
            # reduces. ACT's copies strictly alternate A,B,A,B so each slot's
            # PE refill hides under the other slot's copy: no ACT stalls.
            # Slab cols per tile (PE order):
            #   t0 A [0,1536)->cb[0:1536)      t1 B [1536,3072)->cb[1536:3072)
            #   t2 C [3072,4096)->direct[0:64) t3 A [4096,5632)->cb[3072:4608)
            #   t4 C [5632,6656)->direct[64:)  t5 B [6656,8192)->cb[4608:6144)
            # Trees for block m run during block m+1; L3/L4 per block PAIR.
            PA = psump.tile([P, 1536], F32, space="PSUM", name="PA")
            PB = psump.tile([P, 1536], F32, space="PSUM", name="PB")
            PC = psump.tile([P, 1024], F32, space="PSUM", name="PC")
            cb = scp.tile([P, 2 * CTREE], F16, name="cbpair")
            t1 = trp.tile([P, CTREE], F16, name="t1pair")
            t2 = trp.tile([P, CTREE // 2], F16, name="t2pair")
            t3p = trp.tile([P, CTREE // 4], F16, name="t3pair")
            HL2 = CTREE // 4

            TILES = [  # (psum slot, slab col base, width, consumer, arg)
                # C tiles last: their DVE consumers may lag without ever
                # queueing the next block's A/B fills behind them on the PE.
                # (A/B at 2048/1024 mixed widths measures WORSE: 396.7us --
                # the 2048 refill doesn't hide under the 1024 copy once sem
                # latency is counted. Keep the balanced 1536/1536 slots.)
                (PA, 0, 1536, "act", 0),
                (PB, 1536, 1536, "act", 1536),
                (PA, 3072, 1536, "act", 3072),
                (PB, 4608, 1536, "act", 4608),
                (PC, 6144, 1024, "dve", 0),
                (PC, 7168, 1024, "dve", 64),
            ]

            def emit_tree(m):
                par = m % 2
                cbm = cb[:, par * CTREE : (par + 1) * CTREE]
                h = CTREE // 2
                nc.vector.tensor_max(
                    t1[:, par * h : (par + 1) * h], cbm[:, 0:h], cbm[:, h : 2 * h]
                )
                if par == 1:
                    # L2..L4 for blocks m-1, m in one instr per level (3D APs)
                    q = CTREE // 4
                    t1v = t1[:].rearrange("p (b x) -> p b x", b=2)
                    t2v = t2[:].rearrange("p (b x) -> p b x", b=2)
                    t3v = t3p[:].rearrange("p (b x) -> p b x", b=2)
                    nc.vector.tensor_max(
                        t2v[:, :, :], t1v[:, :, 0:q], t1v[:, :, q : 2 * q]
                    )
                    q //= 2
                    nc.vector.tensor_max(
                        t3v[:, :, :], t2v[:, :, 0:q], t2v[:, :, q : 2 * q]
                    )
                    rv = roots[:, (m - 1) * RPB : (m + 1) * RPB].rearrange(
                        "p (b x) -> p b x", b=2
                    )
                    nc.vector.tensor_max(
                        rv[:, :, 0:NTR], t3v[:, :, 0 : q // 2],
                        t3v[:, :, q // 2 : q],
                    )
                if (m + 1) in DMA_BOUNDS:
                    d0 = DMA_BOUNDS[m + 1] * RPB
                    d1 = (m + 1) * RPB
                    nc.sync.dma_start(out=roots_d[:, d0:d1], in_=roots[:, d0:d1])

            for m in range(mb):
                s = m * RPB
                par = m % 2
                for slot, k0, w, cons, arg in TILES:
                    for cc in range(w // 512):
                        nc.tensor.matmul(
                            out=slot[:, cc * 512 : (cc + 1) * 512],
                            lhsT=t3[:, :, m * P : (m + 1) * P],
                            rhs=w3[:, :, k0 + cc * 512 : k0 + (cc + 1) * 512],
                            start=True,
                            stop=True,
                            perf_mode=mybir.MatmulPerfMode.DoubleRow,
                        )
                    if cons == "act":
                        nc.scalar.activation(
                            cb[:, par * CTREE + arg : par * CTREE + arg + w],
                            slot[:], AF.Copy, bias=0.0,
                        )
                    else:
                        # no high_priority: promoting the direct reduces
                        # delays tree work that gates ACT's cb reuse
                        # (measured +2.7us with the hint)
                        pq3 = slot[:].rearrange("p (j c) -> p j c", c=G)
                        nc.vector.tensor_reduce(
                            out=roots[
                                :, s + NTR + arg : s + NTR + arg + w // G
                            ],
                            in_=pq3[:, :, :],
                            axis=AX.X,
                            op=ALU.max,
                        )
                if m > 0:
                    emit_tree(m - 1)
            emit_tree(mb - 1)

    nc.compile()
    return nc


_CACHE = {}
LAST_RESULT = None
LAST_AMB = -1

# group -> within-slab column offsets [RPB, G]. Slab cols [0, CTREE) are
# ACT-copied in order (cb col == slab col); tree root j = max over cb cols
# {j + NTR*m}. Direct-reduced cols are [CTREE, KC), consecutive 16s.
_COLS = np.empty((RPB, G), np.int64)
_COLS[:NTR] = np.arange(NTR)[:, None] + NTR * np.arange(G)[None, :]
_COLS[NTR:] = CTREE + (np.arange(NDR) * G)[:, None] + np.arange(G)[None, :]


def _get_nc():
    if "nc" not in _CACHE:
        nc = bacc.Bacc(
            "TRN2", target_bir_lowering=False, debug=False, enable_asserts=False
        )
        build_core_kernel(nc)
        _CACHE["nc"] = nc
    return _CACHE["nc"]


def _to_f8_layout(x, width):
    """[256, width] fp32 -> fp8 e4m3 [128, 2*width] d-half-major."""
    x8 = np.clip(x, -224.0, 224.0).astype(ml_dtypes.float8_e4m3)
    return np.ascontiguousarray(
        x8.reshape(2, P, width).transpose(1, 0, 2).reshape(P, 2 * width)
    )


def _prep_weights(W):
    key = (
        W.shape,
        float(W[0, 0]),
        float(W[-1, -1]),
        float(W[::97, ::1013].sum()),
    )
    cached = _CACHE.get("wprep")
    if cached is not None and cached[0] == key:
        return cached[1]
    coln = np.linalg.norm(W.astype(np.float64), axis=0)
    Wu = (W / np.maximum(coln, 1e-30)[None, :]).astype(np.float32)  # [D, K]
    slabs = [
        _to_f8_layout(Wu[:, c * KC : (c + 1) * KC] * SCALE, KC)
        for c in range(NCORES)
    ]
    WT = np.ascontiguousarray(W.T)  # [K, D] fp32
    out = (slabs, coln, WT)
    _CACHE["wprep"] = (key, out)
    return out


def kernel(targ: np.ndarray, W: np.ndarray) -> np.ndarray:
    assert targ.shape == (B, D) and W.shape == (D, K)
    targ = np.ascontiguousarray(targ, dtype=np.float32)
    W = np.ascontiguousarray(W, dtype=np.float32)
    nc = _get_nc()

    slabs, coln, WT = _prep_weights(W)
    rown = np.linalg.norm(targ.astype(np.float64), axis=1)
    tn = (targ / np.maximum(rown, 1e-30)[:, None]).astype(np.float32)  # [B, D]
    tT8 = _to_f8_layout(tn.T * SCALE, B)
    in_maps = [{"tT": tT8, "wn": slabs[c]} for c in range(NCORES)]

    global LAST_RESULT
    LAST_RESULT = run_bass_kernel_spmd(nc, in_maps, list(range(NCORES)))
    res = LAST_RESULT.results

    # roots [128, MB*RPB] -> [B, RPB] with b = m*128 + p, then concat cores
    def unpack(a):
        return a.reshape(P, MB, RPB).transpose(1, 0, 2).reshape(B, RPB)

    flat = np.concatenate(
        [unpack(r["roots"]) for r in res], axis=1
    ).astype(np.float32) / (SCALE * SCALE)              # [B, NCORES*RPB]

    ar = np.arange(B)
    win = np.argmax(flat, axis=1)
    top = flat[ar, win]

    # candidate groups: root within BAND of the top root
    cand_mask = flat >= (top[:, None] - BAND)
    rows, grps = np.nonzero(cand_mask)                  # sorted by (row, grp)
    global LAST_AMB
    LAST_AMB = len(rows)

    # exact rescore of every candidate group, replicating the reference's
    # sim formula (t @ W) / (||t||*||w|| + eps) in float64
    core, r = grps // RPB, grps % RPB
    cand_k = core[:, None] * KC + _COLS[r]              # [N, G] global k
    t64 = targ.astype(np.float64)
    cw = WT[cand_k]                                     # [N, G, 256] fp32
    dots = np.einsum("ngd,nd->ng", cw.astype(np.float64), t64[rows])
    sims = dots / (rown[rows, None] * coln[cand_k] + EPS)

    # per-row argmax over all candidate codes; ties -> smallest k (np.argmax)
    fv = sims.reshape(-1)
    fk = cand_k.reshape(-1)
    fr = np.repeat(rows, G)
    o2 = np.lexsort((fk, -fv, fr))                      # row asc, val desc, k asc
    _, first = np.unique(fr[o2], return_index=True)
    sel = o2[first]
    best_k = np.zeros(B, np.int64)
    best_k[fr[sel]] = fk[sel]

    return WT[best_k].astype(np.float32)
